# revision 22
# baseline (speedup 1.0000x reference)
"""Causal attention head (B=4, S=4096, D=512, E=64) on 8 TRN2 NeuronCores.

Sharding: per batch b, core pair (2b, 2b+1).
 - Even core projects K/V rows [0,2048), queries chunks [0,1024)+[3072,4096).
 - Odd  core projects K/V rows [2048,4096), queries [1024,3072).
 - Projected K^T / V (bf16) exchanged within the pair via AllGather.
 - Flash-style attention with transposed scores S^T = K_chunk @ Q^T, exp on
   ScalarE, PV accumulated with a ones-column appended to V so the softmax
   denominator falls out of the same matmul.
 - Zig-zag query assignment balances causal FLOPs; the two per-parity loop
   structures are selected at runtime with tc.If on a per-core flag.
All matmul inputs are bf16 (inputs pre-cast on host; X^T obtained directly
with DMA-transpose from DRAM). Output f32.
"""

import sys

sys.path.insert(0, "/opt/trn_rl_repo")

import numpy as np
import ml_dtypes

from concourse import bacc, mybir
from concourse import tile
from concourse.bass_utils import run_bass_kernel_spmd

BF16 = ml_dtypes.bfloat16
F32 = mybir.dt.float32
BF = mybir.dt.bfloat16
I32 = mybir.dt.int32

B, S, D, E = 4, 4096, 512, 64
P = 128
NQ = 2048          # queries per core
NKV = 2048         # locally projected kv rows per core
QBLK = 512         # query block
NCHUNK_D = D // P  # 4 contraction chunks for projections
NKCH = S // P      # 32 key chunks in the full sequence
QSTARTS = {0: [0, 1024, 2048, 3072], 1: [512, 1536, 2560, 3584]}
SLOT_J = [8, 16, 24, 32]  # uniform per-slot key-chunk counts (all cores)

_CACHE = {}
LAST_RESULT = None


def _build():
    nc = bacc.Bacc(
        "TRN2",
        target_bir_lowering=False,
        debug=False,
        enable_asserts=True,
        num_devices=8,
    )

    xqt_d = nc.declare_dram_parameter("xqt", [D, NQ], BF, isOutput=False)
    xkt_d = nc.declare_dram_parameter("xkt", [D, S], BF, isOutput=False)
    xvt_d = nc.declare_dram_parameter("xvt", [D, S], BF, isOutput=False)
    wq = nc.declare_dram_parameter("wq", [D, E], BF, isOutput=False)  # pre-scaled 1/8
    wk = nc.declare_dram_parameter("wk", [D, E], BF, isOutput=False)
    wv = nc.declare_dram_parameter("wv", [D, E], BF, isOutput=False)
    masks = nc.declare_dram_parameter("masks", [P, 8 * QBLK], BF, isOutput=False)
    ident = nc.declare_dram_parameter("ident", [P, P], F32, isOutput=False)
    zout = nc.declare_dram_parameter("z", [NQ, E], F32, isOutput=True)

    with tile.TileContext(nc) as tc:
        with (
            tc.tile_pool(name="const", bufs=1) as const,
            tc.tile_pool(name="xt", bufs=1) as xt,
            tc.tile_pool(name="proj", bufs=1) as proj,
            tc.tile_pool(name="work", bufs=4) as work,
            tc.tile_pool(name="epi", bufs=2) as epi,
            tc.tile_pool(name="dram", bufs=1, space="DRAM") as dram,
            tc.tile_pool(name="psA", bufs=2, space="PSUM") as psA,
            tc.tile_pool(name="psZ", bufs=2, space="PSUM") as psZ,
            tc.tile_pool(name="psB", bufs=2, space="PSUM") as psB,
        ):
            # ---- constants ----
            wq_sb = const.tile([P, NCHUNK_D, E], BF, tag="wq")
            wk_sb = const.tile([P, NCHUNK_D, E], BF, tag="wk")
            wv_sb = const.tile([P, NCHUNK_D, E], BF, tag="wv")
            for w_dram, w_sb in ((wq, wq_sb), (wk, wk_sb), (wv, wv_sb)):
                nc.sync.dma_start(
                    out=w_sb[:, :, :],
                    in_=w_dram.rearrange("(c p) e -> p c e", p=P),
                )

            # ---- X^T loads (pre-transposed on host): plain contiguous DMAs ----
            xqt = xt.tile([P, NCHUNK_D * NQ], BF, tag="xqt")
            xkt = xt.tile([P, NCHUNK_D * S], BF, tag="xkt")
            xvt = xt.tile([P, NCHUNK_D * S], BF, tag="xvt")
            for src_d, dst, nr in ((xkt_d, xkt, S), (xqt_d, xqt, NQ), (xvt_d, xvt, S)):
                nseg = nr // 1024
                for s in range(nseg):
                    nc.sync.dma_start(
                        out=dst[:, :]
                        .rearrange("p (c r) -> p c r", c=NCHUNK_D)[
                            :, :, s * 1024 : (s + 1) * 1024
                        ],
                        in_=src_d[:, s * 1024 : (s + 1) * 1024].rearrange(
                            "(c p) r -> p c r", p=P
                        ),
                    )

            # ---- local projections for the FULL sequence (no exchange) ----
            qt2 = proj.tile([P, NQ], BF, tag="qt")
            kt2 = proj.tile([P, S], BF, tag="ktf")
            vp = proj.tile([P, NKCH, E + 1], BF, tag="vp")
            nc.gpsimd.memset(vp[:, :, E : E + 1], 1.0)

            for g in range(S // QBLK):
                ps = psA.tile([E, QBLK], F32, tag="st")
                for c in range(NCHUNK_D):
                    nc.tensor.matmul(
                        ps,
                        lhsT=wk_sb[:, c, :],
                        rhs=xkt[:, c * S + g * QBLK : c * S + (g + 1) * QBLK],
                        start=(c == 0),
                        stop=(c == NCHUNK_D - 1),
                    )
                nc.scalar.activation(
                    out=kt2[0:E, g * QBLK : (g + 1) * QBLK],
                    in_=ps,
                    func=mybir.ActivationFunctionType.Copy,
                )
                nc.vector.tensor_copy(kt2[E : 2 * E, g * QBLK : (g + 1) * QBLK], ps)

            for g in range(NQ // QBLK):
                ps = psA.tile([E, QBLK], F32, tag="st")
                for c in range(NCHUNK_D):
                    nc.tensor.matmul(
                        ps,
                        lhsT=wq_sb[:, c, :],
                        rhs=xqt[:, c * NQ + g * QBLK : c * NQ + (g + 1) * QBLK],
                        start=(c == 0),
                        stop=(c == NCHUNK_D - 1),
                    )
                nc.scalar.activation(
                    out=qt2[0:E, g * QBLK : (g + 1) * QBLK],
                    in_=ps,
                    func=mybir.ActivationFunctionType.Copy,
                )
                nc.vector.tensor_copy(qt2[E : 2 * E, g * QBLK : (g + 1) * QBLK], ps)

            for i in range(S // P):
                ps = psA.tile([P, E], F32, tag="st")
                for c in range(NCHUNK_D):
                    nc.tensor.matmul(
                        ps,
                        lhsT=xvt[:, c * S + i * P : c * S + (i + 1) * P],
                        rhs=wv_sb[:, c, :],
                        start=(c == 0),
                        stop=(c == NCHUNK_D - 1),
                    )
                nc.vector.tensor_copy(vp[:, i, 0:E], ps)

            masks_sb = const.tile([P, 8 * QBLK], BF, tag="masks")
            nc.sync.dma_start(out=masks_sb[:, :], in_=masks[:, :])
            ident_sb = const.tile([P, P], F32, tag="ident")
            nc.sync.dma_start(out=ident_sb[:, :], in_=ident[:, :])

            # ---- attention: software-pipelined, chunk pairs ----
            def attn_block(ib):
                jmax = SLOT_J[ib]
                qloc = ib * QBLK
                zps = psZ.tile([E + 1, QBLK], F32, tag="zt")

                def emit_pv(pt, jp):
                    for h in range(2):
                        j = 2 * jp + h
                        nc.tensor.matmul(
                            zps,
                            lhsT=vp[:, j, :],
                            rhs=pt[:, h * QBLK : (h + 1) * QBLK],
                            start=(j == 0),
                            stop=(j == jmax - 1),
                            skip_group_check=True,
                        )

                prev = None
                for jp in range(jmax // 2):
                    sps = psA.tile([P, 2 * QBLK], F32, tag="st")
                    for h in range(2):
                        j = 2 * jp + h
                        nc.tensor.matmul(
                            sps[:, h * QBLK : (h + 1) * QBLK],
                            lhsT=kt2[h * E : (h + 1) * E, j * P : (j + 1) * P],
                            rhs=qt2[h * E : (h + 1) * E, qloc : qloc + QBLK],
                            start=True,
                            stop=True,
                            tile_position=(h * E, 0),
                        )
                    pt = work.tile([P, 2 * QBLK], BF, tag="pt")
                    nc.scalar.activation(
                        out=pt, in_=sps, func=mybir.ActivationFunctionType.Exp
                    )
                    j0 = 2 * jp
                    if j0 >= jmax - 8:
                        m = j0 - (jmax - 8)
                        nc.vector.tensor_mul(
                            pt, pt, masks_sb[:, m * QBLK : (m + 2) * QBLK]
                        )
                    if prev is not None:
                        emit_pv(*prev)
                    prev = (pt, jp)
                emit_pv(*prev)

                zsb = epi.tile([E + 1, QBLK], F32, tag="zsb")
                nc.vector.tensor_copy(zsb, zps)
                for u in range(QBLK // P):
                    zbp = psB.tile([P, E + 1], F32, tag="zb")
                    nc.tensor.transpose(
                        zbp,
                        zsb[:, u * P : (u + 1) * P],
                        ident_sb[0 : E + 1, 0 : E + 1],
                    )
                    rc = epi.tile([P, 1], F32, tag="rc")
                    nc.vector.reciprocal(rc, zbp[:, E : E + 1])
                    zf = epi.tile([P, E], F32, tag="zf")
                    nc.vector.tensor_scalar_mul(zf, zbp[:, 0:E], rc)
                    row0 = qloc + u * P
                    nc.sync.dma_start(out=zout[row0 : row0 + P, :], in_=zf)

            for ib in range(4):
                attn_block(ib)

    nc.compile()
    return nc


def _get_nc():
    if "nc" not in _CACHE:
        _CACHE["nc"] = _build()
    return _CACHE["nc"]


def _ensure_ntff_hook():
    """Install antenv.axon_hooks + NTFF profile hook if the image lacks it."""
    import types

    try:
        from antenv import axon_hooks  # noqa: F401

        return
    except ImportError:
        pass
    import antenv
    from concourse import bass_utils as _bu

    mod = types.ModuleType("antenv.axon_hooks")
    _state = {}
    mod.set_axon_ntff_profile_hook = lambda h: _state.__setitem__("h", h)
    mod.get_axon_ntff_profile_hook = lambda: _state.get("h")
    sys.modules["antenv.axon_hooks"] = mod
    antenv.axon_hooks = mod
    sys.path.insert(0, "/root/.axon_site/trn_agent_boot")
    from trn_boot import _ntff_profile_via_ctypes

    mod.set_axon_ntff_profile_hook(
        _ntff_profile_via_ctypes("/opt/axon/libaxon_pjrt.so")
    )
    _bu.upload_artifacts = lambda tmpdir: f"local://{tmpdir}"


def _make_masks(h):
    kl = np.arange(P)[:, None]
    ql = np.arange(QBLK)[None, :]
    diag = [(kl <= ql - P * t).astype(np.float32) for t in range(4)]
    ones = np.ones((P, QBLK), np.float32)
    zero = np.zeros((P, QBLK), np.float32)
    tiles = diag + [zero] * 4 if h == 0 else [ones] * 4 + diag
    return np.concatenate(tiles, axis=1).astype(BF16)


def kernel(key_inputs, value_inputs, query_inputs, Wq, Wk, Wv):
    global LAST_RESULT
    import os

    key_inputs = np.asarray(key_inputs, dtype=np.float32)
    value_inputs = np.asarray(value_inputs, dtype=np.float32)
    query_inputs = np.asarray(query_inputs, dtype=np.float32)
    wq_b = (np.asarray(Wq, dtype=np.float32) * 0.125).astype(BF16)
    wk_b = np.asarray(Wk, dtype=np.float32).astype(BF16)
    wv_b = np.asarray(Wv, dtype=np.float32).astype(BF16)
    masks_np = [_make_masks(0), _make_masks(1)]
    ident_np = np.eye(P, dtype=np.float32)

    in_maps = []
    for c in range(8):
        b, h = c // 2, c % 2
        xq_c = np.concatenate(
            [query_inputs[b, q0 : q0 + QBLK] for q0 in QSTARTS[h]], axis=0
        )
        xk_c = key_inputs[b]
        xv_c = value_inputs[b]
        in_maps.append(
            {
                "xqt": np.ascontiguousarray(xq_c.T).astype(BF16),
                "xkt": np.ascontiguousarray(xk_c.T).astype(BF16),
                "xvt": np.ascontiguousarray(xv_c.T).astype(BF16),
                "wq": wq_b,
                "wk": wk_b,
                "wv": wv_b,
                "masks": masks_np[h],
                "ident": ident_np,
            }
        )

    nc = _get_nc()
    trace = bool(int(os.environ.get("KERNEL_TRACE", "0")))
    if trace:
        _ensure_ntff_hook()
    res = run_bass_kernel_spmd(
        nc,
        in_maps,
        core_ids=list(range(8)),
        trace=trace,
        tmpdir=os.environ.get("KERNEL_TRACE_DIR") or None,
    )
    LAST_RESULT = res

    out = np.empty((B, S, E), dtype=np.float32)
    for c in range(8):
        b, h = c // 2, c % 2
        z = np.asarray(res.results[c]["z"], dtype=np.float32)
        for ib, q0 in enumerate(QSTARTS[h]):
            out[b, q0 : q0 + QBLK] = z[ib * QBLK : (ib + 1) * QBLK]
    return out


# revision 23
# speedup vs baseline: 1.0096x; 1.0096x over previous
"""Causal attention head (B=4, S=4096, D=512, E=64) on 8 TRN2 NeuronCores.

Sharding: per batch b, core pair (2b, 2b+1).
 - Even core projects K/V rows [0,2048), queries chunks [0,1024)+[3072,4096).
 - Odd  core projects K/V rows [2048,4096), queries [1024,3072).
 - Projected K^T / V (bf16) exchanged within the pair via AllGather.
 - Flash-style attention with transposed scores S^T = K_chunk @ Q^T, exp on
   ScalarE, PV accumulated with a ones-column appended to V so the softmax
   denominator falls out of the same matmul.
 - Zig-zag query assignment balances causal FLOPs; the two per-parity loop
   structures are selected at runtime with tc.If on a per-core flag.
All matmul inputs are bf16 (inputs pre-cast on host; X^T obtained directly
with DMA-transpose from DRAM). Output f32.
"""

import sys

sys.path.insert(0, "/opt/trn_rl_repo")

import numpy as np
import ml_dtypes

from concourse import bacc, mybir
from concourse import tile
from concourse.bass_utils import run_bass_kernel_spmd

BF16 = ml_dtypes.bfloat16
F32 = mybir.dt.float32
BF = mybir.dt.bfloat16
I32 = mybir.dt.int32

B, S, D, E = 4, 4096, 512, 64
P = 128
NQ = 2048          # queries per core
NKV = 2048         # locally projected kv rows per core
QBLK = 512         # query block
NCHUNK_D = D // P  # 4 contraction chunks for projections
NKCH = S // P      # 32 key chunks in the full sequence
QSTARTS = {0: [0, 1024, 2048, 3072], 1: [512, 1536, 2560, 3584]}
SLOT_J = [8, 16, 24, 32]  # uniform per-slot key-chunk counts (all cores)

_CACHE = {}
LAST_RESULT = None


def _build():
    nc = bacc.Bacc(
        "TRN2",
        target_bir_lowering=False,
        debug=False,
        enable_asserts=True,
        num_devices=8,
    )

    xqt_d = nc.declare_dram_parameter("xqt", [D, NQ], BF, isOutput=False)
    xkt_d = nc.declare_dram_parameter("xkt", [D, S], BF, isOutput=False)
    xvt_d = nc.declare_dram_parameter("xvt", [D, S], BF, isOutput=False)
    wq = nc.declare_dram_parameter("wq", [D, E], BF, isOutput=False)  # pre-scaled 1/8
    wk = nc.declare_dram_parameter("wk", [D, E], BF, isOutput=False)
    wv = nc.declare_dram_parameter("wv", [D, E], BF, isOutput=False)
    masks = nc.declare_dram_parameter("masks", [P, 8 * QBLK], BF, isOutput=False)
    ident = nc.declare_dram_parameter("ident", [P, P], F32, isOutput=False)
    zout = nc.declare_dram_parameter("z", [NQ, E], F32, isOutput=True)

    with tile.TileContext(nc) as tc:
        with (
            tc.tile_pool(name="const", bufs=1) as const,
            tc.tile_pool(name="xt", bufs=1) as xt,
            tc.tile_pool(name="proj", bufs=1) as proj,
            tc.tile_pool(name="work", bufs=3) as work,
            tc.tile_pool(name="epi", bufs=2) as epi,
            tc.tile_pool(name="dram", bufs=1, space="DRAM") as dram,
            tc.tile_pool(name="psA", bufs=2, space="PSUM") as psA,
            tc.tile_pool(name="psZ", bufs=2, space="PSUM") as psZ,
            tc.tile_pool(name="psB", bufs=2, space="PSUM") as psB,
        ):
            # ---- constants ----
            wq_sb = const.tile([P, NCHUNK_D, E], BF, tag="wq")
            wk_sb = const.tile([P, NCHUNK_D, E], BF, tag="wk")
            wv_sb = const.tile([P, NCHUNK_D, E], BF, tag="wv")
            for w_dram, w_sb in ((wq, wq_sb), (wk, wk_sb), (wv, wv_sb)):
                nc.sync.dma_start(
                    out=w_sb[:, :, :],
                    in_=w_dram.rearrange("(c p) e -> p c e", p=P),
                )

            # ---- X^T loads (pre-transposed on host): plain contiguous DMAs ----
            xqt = xt.tile([P, NCHUNK_D * NQ], BF, tag="xqt")
            xkt = xt.tile([P, NCHUNK_D * S], BF, tag="xkt")
            xvt = xt.tile([P, NCHUNK_D * S], BF, tag="xvt")
            for src_d, dst, nr in ((xkt_d, xkt, S), (xqt_d, xqt, NQ), (xvt_d, xvt, S)):
                nseg = nr // 1024
                for s in range(nseg):
                    nc.sync.dma_start(
                        out=dst[:, :]
                        .rearrange("p (c r) -> p c r", c=NCHUNK_D)[
                            :, :, s * 1024 : (s + 1) * 1024
                        ],
                        in_=src_d[:, s * 1024 : (s + 1) * 1024].rearrange(
                            "(c p) r -> p c r", p=P
                        ),
                    )

            # ---- local projections for the FULL sequence (no exchange) ----
            qt2 = proj.tile([P, NQ], BF, tag="qt")
            kt2 = proj.tile([P, S], BF, tag="ktf")
            vp = proj.tile([P, NKCH, E + 1], BF, tag="vp")
            nc.gpsimd.memset(vp[:, :, E : E + 1], 1.0)

            for g in range(S // QBLK):
                ps = psA.tile([E, QBLK], F32, tag="st")
                for c in range(NCHUNK_D):
                    nc.tensor.matmul(
                        ps,
                        lhsT=wk_sb[:, c, :],
                        rhs=xkt[:, c * S + g * QBLK : c * S + (g + 1) * QBLK],
                        start=(c == 0),
                        stop=(c == NCHUNK_D - 1),
                    )
                nc.scalar.activation(
                    out=kt2[0:E, g * QBLK : (g + 1) * QBLK],
                    in_=ps,
                    func=mybir.ActivationFunctionType.Copy,
                )
                nc.vector.tensor_copy(kt2[E : 2 * E, g * QBLK : (g + 1) * QBLK], ps)

            for g in range(NQ // QBLK):
                ps = psA.tile([E, QBLK], F32, tag="st")
                for c in range(NCHUNK_D):
                    nc.tensor.matmul(
                        ps,
                        lhsT=wq_sb[:, c, :],
                        rhs=xqt[:, c * NQ + g * QBLK : c * NQ + (g + 1) * QBLK],
                        start=(c == 0),
                        stop=(c == NCHUNK_D - 1),
                    )
                nc.scalar.activation(
                    out=qt2[0:E, g * QBLK : (g + 1) * QBLK],
                    in_=ps,
                    func=mybir.ActivationFunctionType.Copy,
                )
                nc.vector.tensor_copy(qt2[E : 2 * E, g * QBLK : (g + 1) * QBLK], ps)

            for i in range(S // P):
                ps = psA.tile([P, E], F32, tag="st")
                for c in range(NCHUNK_D):
                    nc.tensor.matmul(
                        ps,
                        lhsT=xvt[:, c * S + i * P : c * S + (i + 1) * P],
                        rhs=wv_sb[:, c, :],
                        start=(c == 0),
                        stop=(c == NCHUNK_D - 1),
                    )
                nc.vector.tensor_copy(vp[:, i, 0:E], ps)

            masks_sb = const.tile([P, 8 * QBLK], BF, tag="masks")
            nc.sync.dma_start(out=masks_sb[:, :], in_=masks[:, :])
            ident_sb = const.tile([P, P], F32, tag="ident")
            nc.sync.dma_start(out=ident_sb[:, :], in_=ident[:, :])

            # ---- attention: software-pipelined, chunk pairs ----
            def attn_block(ib):
                jmax = SLOT_J[ib]
                qloc = ib * QBLK
                zps = psZ.tile([E + 1, QBLK], F32, tag="zt")

                def emit_pv(pt, jp):
                    for h in range(2):
                        j = 2 * jp + h
                        nc.tensor.matmul(
                            zps,
                            lhsT=vp[:, j, :],
                            rhs=pt[:, h * QBLK : (h + 1) * QBLK],
                            start=(j == 0),
                            stop=(j == jmax - 1),
                            skip_group_check=True,
                        )

                prev = None
                for jp in range(jmax // 2):
                    sps = psA.tile([P, 2 * QBLK], F32, tag="st")
                    for h in range(2):
                        j = 2 * jp + h
                        nc.tensor.matmul(
                            sps[:, h * QBLK : (h + 1) * QBLK],
                            lhsT=kt2[h * E : (h + 1) * E, j * P : (j + 1) * P],
                            rhs=qt2[h * E : (h + 1) * E, qloc : qloc + QBLK],
                            start=True,
                            stop=True,
                            tile_position=(h * E, 0),
                        )
                    pt = work.tile([P, 2 * QBLK], BF, tag="pt")
                    nc.scalar.activation(
                        out=pt, in_=sps, func=mybir.ActivationFunctionType.Exp
                    )
                    j0 = 2 * jp
                    if j0 >= jmax - 8:
                        m = j0 - (jmax - 8)
                        nc.vector.tensor_mul(
                            pt, pt, masks_sb[:, m * QBLK : (m + 2) * QBLK]
                        )
                    if prev is not None:
                        emit_pv(*prev)
                    prev = (pt, jp)
                emit_pv(*prev)

                zsb = epi.tile([E + 1, QBLK], F32, tag="zsb")
                nc.vector.tensor_copy(zsb, zps)
                for u in range(QBLK // P):
                    zbp = psB.tile([P, E + 1], F32, tag="zb")
                    nc.tensor.transpose(
                        zbp,
                        zsb[:, u * P : (u + 1) * P],
                        ident_sb[0 : E + 1, 0 : E + 1],
                    )
                    rc = epi.tile([P, 1], F32, tag="rc")
                    nc.vector.reciprocal(rc, zbp[:, E : E + 1])
                    zf = epi.tile([P, E], F32, tag="zf")
                    nc.vector.tensor_scalar_mul(zf, zbp[:, 0:E], rc)
                    row0 = qloc + u * P
                    nc.sync.dma_start(out=zout[row0 : row0 + P, :], in_=zf)

            for ib in range(4):
                attn_block(ib)

    nc.compile()
    return nc


def _get_nc():
    if "nc" not in _CACHE:
        _CACHE["nc"] = _build()
    return _CACHE["nc"]


def _ensure_ntff_hook():
    """Install antenv.axon_hooks + NTFF profile hook if the image lacks it."""
    import types

    try:
        from antenv import axon_hooks  # noqa: F401

        return
    except ImportError:
        pass
    import antenv
    from concourse import bass_utils as _bu

    mod = types.ModuleType("antenv.axon_hooks")
    _state = {}
    mod.set_axon_ntff_profile_hook = lambda h: _state.__setitem__("h", h)
    mod.get_axon_ntff_profile_hook = lambda: _state.get("h")
    sys.modules["antenv.axon_hooks"] = mod
    antenv.axon_hooks = mod
    sys.path.insert(0, "/root/.axon_site/trn_agent_boot")
    from trn_boot import _ntff_profile_via_ctypes

    mod.set_axon_ntff_profile_hook(
        _ntff_profile_via_ctypes("/opt/axon/libaxon_pjrt.so")
    )
    _bu.upload_artifacts = lambda tmpdir: f"local://{tmpdir}"


def _make_masks(h):
    kl = np.arange(P)[:, None]
    ql = np.arange(QBLK)[None, :]
    diag = [(kl <= ql - P * t).astype(np.float32) for t in range(4)]
    ones = np.ones((P, QBLK), np.float32)
    zero = np.zeros((P, QBLK), np.float32)
    tiles = diag + [zero] * 4 if h == 0 else [ones] * 4 + diag
    return np.concatenate(tiles, axis=1).astype(BF16)


def kernel(key_inputs, value_inputs, query_inputs, Wq, Wk, Wv):
    global LAST_RESULT
    import os

    key_inputs = np.asarray(key_inputs, dtype=np.float32)
    value_inputs = np.asarray(value_inputs, dtype=np.float32)
    query_inputs = np.asarray(query_inputs, dtype=np.float32)
    wq_b = (np.asarray(Wq, dtype=np.float32) * 0.125).astype(BF16)
    wk_b = np.asarray(Wk, dtype=np.float32).astype(BF16)
    wv_b = np.asarray(Wv, dtype=np.float32).astype(BF16)
    masks_np = [_make_masks(0), _make_masks(1)]
    ident_np = np.eye(P, dtype=np.float32)

    in_maps = []
    for c in range(8):
        b, h = c // 2, c % 2
        xq_c = np.concatenate(
            [query_inputs[b, q0 : q0 + QBLK] for q0 in QSTARTS[h]], axis=0
        )
        xk_c = key_inputs[b]
        xv_c = value_inputs[b]
        in_maps.append(
            {
                "xqt": np.ascontiguousarray(xq_c.T).astype(BF16),
                "xkt": np.ascontiguousarray(xk_c.T).astype(BF16),
                "xvt": np.ascontiguousarray(xv_c.T).astype(BF16),
                "wq": wq_b,
                "wk": wk_b,
                "wv": wv_b,
                "masks": masks_np[h],
                "ident": ident_np,
            }
        )

    nc = _get_nc()
    trace = bool(int(os.environ.get("KERNEL_TRACE", "0")))
    if trace:
        _ensure_ntff_hook()
    res = run_bass_kernel_spmd(
        nc,
        in_maps,
        core_ids=list(range(8)),
        trace=trace,
        tmpdir=os.environ.get("KERNEL_TRACE_DIR") or None,
    )
    LAST_RESULT = res

    out = np.empty((B, S, E), dtype=np.float32)
    for c in range(8):
        b, h = c // 2, c % 2
        z = np.asarray(res.results[c]["z"], dtype=np.float32)
        for ib, q0 in enumerate(QSTARTS[h]):
            out[b, q0 : q0 + QBLK] = z[ib * QBLK : (ib + 1) * QBLK]
    return out
